# revision 20
# baseline (speedup 1.0000x reference)
"""Causal attention (B=4, S=2048, D=1024) on 8 Trainium2 NeuronCores.

Sharding: data-parallel over batch (4) x query-parity-parallel (2 cores per
batch).  Global q-tiles (128 rows, 16 per batch) are dealt round-robin: core
h=0 of a pair takes even tiles, h=1 odd tiles.

Each core computes K^T, V and Q^T only for its OWN-parity rows; the host
pre-transposes (and pre-casts to bf16) the own-parity activation rows, so
there are NO transposes anywhere on the device.  Projection outputs land
directly in the own (slot 0) half of the kT / v SBUF tiles; only the 2MB K
and 2MB V exchanges bounce through HBM AllGathers, and only the PARTNER
gather slot (picked with a partition_id-driven dynamic DMA offset) is read
back, into slot 1.  The causal asymmetry between the two cores lives
entirely in a per-core additive-mask input (slot 0 diag = triangle, slot 1
diag = all-masked for h=0 / all-kept for h=1).

Attention runs in the TRANSPOSED-scores formulation: for each key tile
(slot, kt) we compute ST[k, q] = (kT tile)^T qT over the query suffix q >=
128*kt, add the mask on the leading 128-col block, and exp straight into
PT[k, q] -- the exact stationary operand the O matmuls need, so the 72
per-tile PE transposes of P vanish.  Row sums come from an extra N=1 matmul
against a ones-vector that reuses the already-loaded PT stationary.  All
matmuls are bf16 with fp32 PSUM accumulation:

  xqT[d, s]  : host-pre-transposed bf16 own-parity rows
  KTo[e, k]  = wk^T xqT,  V[k, e] = xqT^T wv,  QT[e, q] = wq^T xqT / 32
  ST[k, q]   = KT_tile^T QT over the q-suffix (chunks of <=512 cols in PSUM)
  PT         = exp(ST + mask), bf16
  O[q, e]    = sum_tiles PT_tile^T V_tile, scaled by 1/rowsum
"""

import os

os.environ.setdefault("MYCRO_LOCAL_CACHE", "1")

import ml_dtypes
import numpy as np

import concourse.bacc as bacc
import concourse.tile as tile
from concourse import mybir
from concourse.bass import ts
from concourse.bass_utils import run_bass_kernel_spmd

B, S, D = 4, 2048, 1024
P = 128
QL = S // 2          # queries per core == own-parity keys per core
NCORES = 8
DT = D // P          # 8 d-tiles (contraction)
ET = D // P          # 8 e-tiles
NQT = QL // P        # 8 q-tiles per core
NKT = QL // P        # 8 own-parity k-tiles per core
F32 = mybir.dt.float32
BF16 = mybir.dt.bfloat16
NEG = -30000.0       # additive mask value; exp() underflows to exactly 0
PAIRS = [[2 * b, 2 * b + 1] for b in range(B)]


def _off(kt):
    """Column offset of key-tile kt's block inside PT (suffix len 1024-128t)."""
    return kt * QL - P * kt * (kt - 1) // 2


PT_W = _off(NKT)     # 4608


def _body(tc, xq, wq, wk, wv, mask, out):
    nc = tc.nc
    with (
        tc.tile_pool(name="consts", bufs=1) as consts,
        tc.tile_pool(name="qkv", bufs=1) as qkv,
        tc.tile_pool(name="dram", bufs=1, space="DRAM") as dram,
        tc.tile_pool(name="pmm", bufs=3, space="PSUM") as pmm,
    ):
        # ---- constants + HAM warmup (PE would otherwise sit cold during the
        # initial DMA phase and start the projections at 1.2 GHz)
        warm = consts.tile([P, 512], BF16)
        nc.vector.memset(warm, 0.0)
        ones = consts.tile([P, 1], BF16)
        nc.vector.memset(ones, 1.0)
        mask_sb = consts.tile([P, 256], F32)
        nc.sync.dma_start(mask_sb, mask)
        for _ in range(10):
            ps = pmm.tile([P, 512], F32, tag="mm")
            nc.tensor.matmul(ps, warm[:, 0:P], warm, start=True, stop=True)

        # slot semantics: 0 = OWN parity half (written locally, never
        # bounced), 1 = PARTNER half (via exchange + dynamic-slot readback).
        xqT = qkv.tile([P, DT, QL], BF16)       # [d_in, d_tile, s_own]
        qT = qkv.tile([P, ET, QL], BF16)        # [e_in, e_tile, q]
        kT = qkv.tile([P, 2, ET, QL], BF16)     # [e_in, slot, e_tile, k]
        v = qkv.tile([P, 2, NKT, D], BF16)      # [k_in, slot, k_tile, e]
        PT = qkv.tile([P, 2, PT_W], BF16)       # [k_in, slot, packed blocks]

        # HBM bounce buffers for the pair exchanges (slot p = parity p).
        cw_loc = dram.tile([P, 1], F32)
        cw_gth = dram.tile([2, P, 1], F32)
        k_loc = dram.tile([P, ET, QL], BF16)
        k_gth = dram.tile([2, P, ET, QL], BF16)
        v_loc = dram.tile([P, NKT, D], BF16)
        v_gth = dram.tile([2, P, NKT, D], BF16)

        # ---- tiny warmup AllGather: pays the cc-ring bring-up cost (~11us
        # trigger->start lag) before the real exchanges need the stream.
        nc.sync.dma_start(cw_loc, mask[:, 0:1])
        nc.gpsimd.collective_compute(
            "AllGather", mybir.AluOpType.bypass, replica_groups=PAIRS,
            ins=[cw_loc.opt()], outs=[cw_gth.opt()])

        # ------------------------------ projections ------------------------
        with tc.tile_pool(name="wsb", bufs=3) as wpool:
            # xq arrives PRE-TRANSPOSED bf16 from the host ([d, s] layout):
            # plain full-rate DMAs, no on-device transposes anywhere.  The
            # low column halves go first so K chunk 0 unblocks early.
            for half in range(2):
                cols = slice(half * 512, (half + 1) * 512)
                for dd in range(DT):
                    nc.sync.dma_start(xqT[:, dd, cols],
                                      xq[dd * P:(dd + 1) * P, cols])

            # weights arrive bf16: straight HBM->SBUF copies (scalar queue)
            def load_weight(w_ap):
                wsb = wpool.tile([P, DT, D], BF16, tag="w")
                for d in range(DT):
                    nc.scalar.dma_start(wsb[:, d, :], w_ap[d * P:(d + 1) * P, :])
                return wsb

            wk_sb = load_weight(wk)

            # ---- K^T for own-parity keys: PSUM lands straight in the OWN
            # slot of kT; the exchange input is one contiguous 2MB store on
            # the (idle) sync queue.  The wv/wq load triggers are emitted
            # between the copy batches so they never delay a PSUM copy.
            wv_sb = wq_sb = None
            for c in range(QL // 512):
                for e in range(ET):
                    ps = pmm.tile([P, 512], F32, tag="mm")
                    for d in range(DT):
                        nc.tensor.matmul(
                            ps, wk_sb[:, d, e * P:(e + 1) * P],
                            xqT[:, d, c * 512:(c + 1) * 512],
                            start=(d == 0), stop=(d == DT - 1))
                    nc.scalar.copy(kT[:, 0, e, c * 512:(c + 1) * 512], ps)
                if c == 0:
                    wv_sb = load_weight(wv)
                else:
                    wq_sb = load_weight(wq)
            nc.sync.dma_start(k_loc, kT[:, 0, :, :])

            # ---- pair K exchange: the collective instruction is a
            # non-blocking doorbell (consumers wait on its completion
            # semaphore), so both exchange triggers fire at their data-ready
            # times and the transfers pipeline on the cc stream.
            nc.gpsimd.collective_compute(
                "AllGather", mybir.AluOpType.bypass, replica_groups=PAIRS,
                ins=[k_loc.opt()], outs=[k_gth.opt()])

            # ---- V for own-parity keys
            for kt in range(NKT):
                for ec in range(D // 512):
                    ps = pmm.tile([P, 512], F32, tag="mm")
                    for d in range(DT):
                        nc.tensor.matmul(
                            ps, xqT[:, d, kt * P:(kt + 1) * P],
                            wv_sb[:, d, ec * 512:(ec + 1) * 512],
                            start=(d == 0), stop=(d == DT - 1))
                    nc.scalar.copy(v[:, 0, kt, ec * 512:(ec + 1) * 512], ps)
            nc.sync.dma_start(v_loc, v[:, 0, :, :])
            nc.gpsimd.collective_compute(
                "AllGather", mybir.AluOpType.bypass, replica_groups=PAIRS,
                ins=[v_loc.opt()], outs=[v_gth.opt()])

            # ---- partner-half readbacks: only gather slot (1-h) is read,
            # via dynamic-offset DMAs split across the gpsimd+vector queues.
            pg = 1 - (nc.gpsimd.partition_id() & 1)
            py = 1 - (nc.sync.partition_id() & 1)
            for i in range(4):
                eng, pp = (nc.gpsimd, pg) if i % 2 == 0 else (nc.sync, py)
                eng.dma_start(kT[:, 1, 2 * i:2 * i + 2, :],
                              k_gth[ts(pp, 1), :, 2 * i:2 * i + 2, :])
            for i in range(4):
                eng, pp = (nc.gpsimd, pg) if i % 2 == 0 else (nc.sync, py)
                eng.dma_start(v[:, 1, 2 * i:2 * i + 2, :],
                              v_gth[ts(pp, 1), :, 2 * i:2 * i + 2, :])

            # ---- Q^T
            for c in range(2):
                for e in range(ET):
                    ps = pmm.tile([P, 512], F32, tag="mm")
                    for d in range(DT):
                        nc.tensor.matmul(
                            ps, wq_sb[:, d, e * P:(e + 1) * P],
                            xqT[:, d, c * 512:(c + 1) * 512],
                            start=(d == 0), stop=(d == DT - 1))
                    nc.scalar.mul(qT[:, e, c * 512:(c + 1) * 512], ps,
                                  1.0 / 32.0)

        # ------------------------------ attention --------------------------
        # S-phase: ST[k, q-suffix] per (slot, kt), exp into PT.  The OWN
        # slot goes first -- it needs no exchange data, so it overlaps the
        # tail of the K exchange + partner readback.
        for p in range(2):
            for kt in range(NKT):
                q0 = kt * P
                col = q0
                while col < QL:
                    cw = min(512, QL - col)
                    ps = pmm.tile([P, cw], F32, tag="mm")
                    for e in range(ET):
                        nc.tensor.matmul(
                            ps, kT[:, p, e, kt * P:(kt + 1) * P],
                            qT[:, e, col:col + cw],
                            start=(e == 0), stop=(e == ET - 1))
                    if col == q0:
                        nc.vector.tensor_add(
                            ps[:, 0:P], ps[:, 0:P],
                            mask_sb[:, p * P:(p + 1) * P])
                    nc.scalar.activation(
                        PT[:, p, _off(kt) + col - q0:_off(kt) + col - q0 + cw],
                        ps, mybir.ActivationFunctionType.Exp)
                    col += cw

        # O-phase: O[q,e] = sum PT_tile^T V_tile; rowsum via an extra N=1
        # matmul on the same stationary.
        with (
            tc.tile_pool(name="psO", bufs=2, space="PSUM") as psO,
            tc.tile_pool(name="psl", bufs=1, space="PSUM") as pslp,
            tc.tile_pool(name="oout", bufs=2) as opool,
            tc.tile_pool(name="stats", bufs=2) as spool,
        ):
            psl = pslp.tile([P, NQT], F32)
            for j in range(NQT):
                po = psO.tile([P, D], F32, tag="o")
                n_units = 2 * (j + 1)
                i = 0
                for kt in range(j + 1):
                    for p in range(2):
                        st = PT[:, p, _off(kt) + (j - kt) * P:
                                _off(kt) + (j - kt) * P + P]
                        for ec in range(D // 512):
                            nc.tensor.matmul(
                                po[:, ec * 512:(ec + 1) * 512], st,
                                v[:, p, kt, ec * 512:(ec + 1) * 512],
                                start=(i == 0), stop=(i == n_units - 1))
                        nc.tensor.matmul(
                            psl[:, j:j + 1], st, ones,
                            start=(i == 0), stop=(i == n_units - 1))
                        i += 1
                linv = spool.tile([P, 1], F32, tag="linv")
                nc.vector.reciprocal(linv, psl[:, j:j + 1])
                o_sb = opool.tile([P, D], F32, tag="o")
                for c in range(D // 512):
                    nc.vector.tensor_scalar_mul(
                        o_sb[:, c * 512:(c + 1) * 512],
                        po[:, c * 512:(c + 1) * 512], linv)
                    nc.sync.dma_start(
                        out[j * P:(j + 1) * P, c * 512:(c + 1) * 512],
                        o_sb[:, c * 512:(c + 1) * 512])


_PROG = None


def _get_prog():
    global _PROG
    if _PROG is None:
        nc = bacc.Bacc("TRN2", target_bir_lowering=False, debug=False,
                       enable_asserts=False)
        xq = nc.dram_tensor("xq", (D, QL), BF16, kind="ExternalInput").ap()
        wq = nc.dram_tensor("wq", (D, D), BF16, kind="ExternalInput").ap()
        wk = nc.dram_tensor("wk", (D, D), BF16, kind="ExternalInput").ap()
        wv = nc.dram_tensor("wv", (D, D), BF16, kind="ExternalInput").ap()
        mask = nc.dram_tensor("mask", (P, 256), F32, kind="ExternalInput").ap()
        out = nc.dram_tensor("out", (QL, D), F32, kind="ExternalOutput").ap()
        with tile.TileContext(nc) as tc:
            _body(tc, xq, wq, wk, wv, mask, out)
        nc.compile()
        _PROG = nc
    return _PROG


def _mask_np(h):
    """[k, q]-layout additive mask: block 0 = OWN slot, block 1 = PARTNER.

    Own slot: transposed causal triangle (keep k <= q) on the diagonal
    tile.  Partner slot diagonal tile: h=0's partner keys are ABOVE the
    diagonal (all masked); h=1's are below (all kept).
    """
    r = np.arange(P)[:, None]   # k (partition)
    c = np.arange(P)[None, :]   # q (free)
    tri = np.where(r <= c, 0.0, NEG).astype(np.float32)
    m = np.zeros((P, 256), np.float32)
    m[:, 0:P] = tri
    if h == 0:
        m[:, P:] = NEG
    return m


def _in_map_for_core(inputs, core):
    b, h = core // 2, core % 2
    xb = np.asarray(inputs["x"], np.float32)[b]
    xqb = xb.reshape(NQT, 2, P, D)[:, h].reshape(QL, D)
    bf = ml_dtypes.bfloat16
    return {
        "xq": np.ascontiguousarray(xqb.T.astype(bf)),
        "wq": np.ascontiguousarray(np.asarray(inputs["wq"]).astype(bf)),
        "wk": np.ascontiguousarray(np.asarray(inputs["wk"]).astype(bf)),
        "wv": np.ascontiguousarray(np.asarray(inputs["wv"]).astype(bf)),
        "mask": _mask_np(h),
    }


def _run(inputs, trace=False, tmpdir=None):
    nc = _get_prog()
    in_maps = [_in_map_for_core(inputs, c) for c in range(NCORES)]
    try:
        res = run_bass_kernel_spmd(nc, in_maps, core_ids=list(range(NCORES)),
                                   trace=trace, tmpdir=tmpdir)
    except Exception:
        # first execution of a fresh NEFF occasionally trips a transient
        # device error on this stack; one retry has always succeeded
        res = run_bass_kernel_spmd(nc, in_maps, core_ids=list(range(NCORES)),
                                   trace=trace, tmpdir=tmpdir)
    outf = np.empty((B, S, D), np.float32)
    for core in range(NCORES):
        b, h = core // 2, core % 2
        o = np.asarray(res.results[core]["out"], np.float32)
        outf[b].reshape(NQT, 2, P, D)[:, h] = o.reshape(NQT, P, D)
    return outf, res


def kernel(x, wq, wk, wv):
    outf, _ = _run({"x": x, "wq": wq, "wk": wk, "wv": wv}, trace=False)
    return outf


# revision 21
# speedup vs baseline: 1.0589x; 1.0589x over previous
"""Causal attention (B=4, S=2048, D=1024) on 8 Trainium2 NeuronCores.

Sharding: data-parallel over batch (4) x query-parity-parallel (2 cores per
batch).  Global q-tiles (128 rows, 16 per batch) are dealt round-robin: core
h=0 of a pair takes even tiles, h=1 odd tiles.

Each core computes K^T, V and Q^T only for its OWN-parity rows; the host
pre-transposes (and pre-casts to bf16) the own-parity activation rows, so
there are NO transposes anywhere on the device.  Projection outputs land
directly in the own (slot 0) half of the kT / v SBUF tiles; only the 2MB K
and 2MB V exchanges bounce through HBM AllGathers, and only the PARTNER
gather slot (picked with a partition_id-driven dynamic DMA offset) is read
back, into slot 1.  The causal asymmetry between the two cores lives
entirely in a per-core additive-mask input (slot 0 diag = triangle, slot 1
diag = all-masked for h=0 / all-kept for h=1).

Attention runs in the TRANSPOSED-scores formulation: for each key tile
(slot, kt) we compute ST[k, q] = (kT tile)^T qT over the query suffix q >=
128*kt, add the mask on the leading 128-col block, and exp straight into
PT[k, q] -- the exact stationary operand the O matmuls need, so the 72
per-tile PE transposes of P vanish.  Row sums come from an extra N=1 matmul
against a ones-vector that reuses the already-loaded PT stationary.  All
matmuls are bf16 with fp32 PSUM accumulation:

  xqT[d, s]  : host-pre-transposed bf16 own-parity rows
  KTo[e, k]  = wk^T xqT,  V[k, e] = xqT^T wv,  QT[e, q] = wq^T xqT / 32
  ST[k, q]   = KT_tile^T QT over the q-suffix (chunks of <=512 cols in PSUM)
  PT         = exp(ST + mask), bf16
  O[q, e]    = sum_tiles PT_tile^T V_tile, scaled by 1/rowsum
"""

import os

os.environ.setdefault("MYCRO_LOCAL_CACHE", "1")

import ml_dtypes
import numpy as np

import concourse.bacc as bacc
import concourse.tile as tile
from concourse import mybir
from concourse.bass import ts
from concourse.bass_utils import run_bass_kernel_spmd

B, S, D = 4, 2048, 1024
P = 128
QL = S // 2          # queries per core == own-parity keys per core
NCORES = 8
DT = D // P          # 8 d-tiles (contraction)
ET = D // P          # 8 e-tiles
NQT = QL // P        # 8 q-tiles per core
NKT = QL // P        # 8 own-parity k-tiles per core
F32 = mybir.dt.float32
BF16 = mybir.dt.bfloat16
NEG = -30000.0       # additive mask value; exp() underflows to exactly 0
PAIRS = [[2 * b, 2 * b + 1] for b in range(B)]


def _off(kt):
    """Column offset of key-tile kt's block inside PT (suffix len 1024-128t)."""
    return kt * QL - P * kt * (kt - 1) // 2


PT_W = _off(NKT)     # 4608


def _body(tc, xq, wq, wk, wv, mask, out):
    nc = tc.nc
    with (
        tc.tile_pool(name="consts", bufs=1) as consts,
        tc.tile_pool(name="qkv", bufs=1) as qkv,
        tc.tile_pool(name="dram", bufs=1, space="DRAM") as dram,
        tc.tile_pool(name="pmm", bufs=3, space="PSUM") as pmm,
    ):
        # ---- constants + HAM warmup (PE would otherwise sit cold during the
        # initial DMA phase and start the projections at 1.2 GHz)
        warm = consts.tile([P, 512], BF16)
        nc.vector.memset(warm, 0.0)
        ones = consts.tile([P, 1], BF16)
        nc.vector.memset(ones, 1.0)
        mask_sb = consts.tile([P, 256], F32)
        nc.sync.dma_start(mask_sb, mask)
        for _ in range(10):
            ps = pmm.tile([P, 512], F32, tag="mm")
            nc.tensor.matmul(ps, warm[:, 0:P], warm, start=True, stop=True)

        # slot semantics: 0 = OWN parity half (written locally, never
        # bounced), 1 = PARTNER half (via exchange + dynamic-slot readback).
        xqT = qkv.tile([P, DT, QL], BF16)       # [d_in, d_tile, s_own]
        qT = qkv.tile([P, ET, QL], BF16)        # [e_in, e_tile, q]
        kT = qkv.tile([P, 2, ET, QL], BF16)     # [e_in, slot, e_tile, k]
        v = qkv.tile([P, 2, NKT, D], BF16)      # [k_in, slot, k_tile, e]
        PT = qkv.tile([P, 2, PT_W], BF16)       # [k_in, slot, packed blocks]

        # HBM bounce buffers for the pair exchanges (slot p = parity p).
        cw_loc = dram.tile([1, 4], F32)
        cw_gth = dram.tile([2, 1, 4], F32)
        k_loc = dram.tile([P, ET, QL], BF16)
        k_gth = dram.tile([2, P, ET, QL], BF16)
        v_loc = dram.tile([P, NKT, D], BF16)
        v_gth = dram.tile([2, P, NKT, D], BF16)

        # ---- tiny warmup AllGather: pays the cc-ring bring-up cost (~11us
        # trigger->start lag) before the real exchanges need the stream.
        nc.gpsimd.dma_start(cw_loc, mask[0:1, 0:4])
        nc.gpsimd.collective_compute(
            "AllGather", mybir.AluOpType.bypass, replica_groups=PAIRS,
            ins=[cw_loc.opt()], outs=[cw_gth.opt()])

        # ------------------------------ projections ------------------------
        with tc.tile_pool(name="wsb", bufs=3) as wpool:
            # xq arrives PRE-TRANSPOSED bf16 from the host ([d, s] layout):
            # plain full-rate DMAs, no on-device transposes anywhere.  The
            # low column halves go first so K chunk 0 unblocks early.
            for half in range(2):
                cols = slice(half * 512, (half + 1) * 512)
                for dd in range(DT):
                    nc.sync.dma_start(xqT[:, dd, cols],
                                      xq[dd * P:(dd + 1) * P, cols])

            # weights arrive bf16: straight HBM->SBUF copies (scalar queue)
            def load_weight(w_ap):
                wsb = wpool.tile([P, DT, D], BF16, tag="w")
                for d in range(DT):
                    nc.scalar.dma_start(wsb[:, d, :], w_ap[d * P:(d + 1) * P, :])
                return wsb

            wk_sb = load_weight(wk)

            # ---- K^T for own-parity keys: PSUM lands straight in the OWN
            # slot of kT; the exchange input is one contiguous 2MB store on
            # the (idle) sync queue.  The wv/wq load triggers are emitted
            # between the copy batches so they never delay a PSUM copy.
            wv_sb = wq_sb = None
            for c in range(QL // 512):
                for e in range(ET):
                    ps = pmm.tile([P, 512], F32, tag="mm")
                    for d in range(DT):
                        nc.tensor.matmul(
                            ps, wk_sb[:, d, e * P:(e + 1) * P],
                            xqT[:, d, c * 512:(c + 1) * 512],
                            start=(d == 0), stop=(d == DT - 1))
                    nc.scalar.copy(kT[:, 0, e, c * 512:(c + 1) * 512], ps)
                if c == 0:
                    wv_sb = load_weight(wv)
                else:
                    wq_sb = load_weight(wq)
            nc.sync.dma_start(k_loc, kT[:, 0, :, :])

            # ---- pair K exchange: the collective instruction is a
            # non-blocking doorbell (consumers wait on its completion
            # semaphore), so both exchange triggers fire at their data-ready
            # times and the transfers pipeline on the cc stream.
            nc.gpsimd.collective_compute(
                "AllGather", mybir.AluOpType.bypass, replica_groups=PAIRS,
                ins=[k_loc.opt()], outs=[k_gth.opt()])

            # ---- V for own-parity keys
            for kt in range(NKT):
                for ec in range(D // 512):
                    ps = pmm.tile([P, 512], F32, tag="mm")
                    for d in range(DT):
                        nc.tensor.matmul(
                            ps, xqT[:, d, kt * P:(kt + 1) * P],
                            wv_sb[:, d, ec * 512:(ec + 1) * 512],
                            start=(d == 0), stop=(d == DT - 1))
                    nc.scalar.copy(v[:, 0, kt, ec * 512:(ec + 1) * 512], ps)
            nc.sync.dma_start(v_loc, v[:, 0, :, :])
            nc.gpsimd.collective_compute(
                "AllGather", mybir.AluOpType.bypass, replica_groups=PAIRS,
                ins=[v_loc.opt()], outs=[v_gth.opt()])

            # ---- partner-half readbacks: only gather slot (1-h) is read,
            # via dynamic-offset DMAs split across the gpsimd+vector queues.
            pg = 1 - (nc.gpsimd.partition_id() & 1)
            py = 1 - (nc.sync.partition_id() & 1)
            for i in range(4):
                eng, pp = (nc.gpsimd, pg) if i % 2 == 0 else (nc.sync, py)
                eng.dma_start(kT[:, 1, 2 * i:2 * i + 2, :],
                              k_gth[ts(pp, 1), :, 2 * i:2 * i + 2, :])
            for i in range(4):
                eng, pp = (nc.gpsimd, pg) if i % 2 == 0 else (nc.sync, py)
                eng.dma_start(v[:, 1, 2 * i:2 * i + 2, :],
                              v_gth[ts(pp, 1), :, 2 * i:2 * i + 2, :])

            # ---- Q^T
            for c in range(2):
                for e in range(ET):
                    ps = pmm.tile([P, 512], F32, tag="mm")
                    for d in range(DT):
                        nc.tensor.matmul(
                            ps, wq_sb[:, d, e * P:(e + 1) * P],
                            xqT[:, d, c * 512:(c + 1) * 512],
                            start=(d == 0), stop=(d == DT - 1))
                    nc.scalar.mul(qT[:, e, c * 512:(c + 1) * 512], ps,
                                  1.0 / 32.0)

        # ------------------------------ attention --------------------------
        # S-phase: ST[k, q-suffix] per (slot, kt), exp into PT.  The OWN
        # slot goes first -- it needs no exchange data, so it overlaps the
        # tail of the K exchange + partner readback.
        for p in range(2):
            for kt in range(NKT):
                q0 = kt * P
                col = q0
                while col < QL:
                    cw = min(512, QL - col)
                    ps = pmm.tile([P, cw], F32, tag="mm")
                    for e in range(ET):
                        nc.tensor.matmul(
                            ps, kT[:, p, e, kt * P:(kt + 1) * P],
                            qT[:, e, col:col + cw],
                            start=(e == 0), stop=(e == ET - 1))
                    if col == q0:
                        nc.vector.tensor_add(
                            ps[:, 0:P], ps[:, 0:P],
                            mask_sb[:, p * P:(p + 1) * P])
                    nc.scalar.activation(
                        PT[:, p, _off(kt) + col - q0:_off(kt) + col - q0 + cw],
                        ps, mybir.ActivationFunctionType.Exp)
                    col += cw

        # O-phase: O[q,e] = sum PT_tile^T V_tile; rowsum via an extra N=1
        # matmul on the same stationary.
        with (
            tc.tile_pool(name="psO", bufs=2, space="PSUM") as psO,
            tc.tile_pool(name="psl", bufs=1, space="PSUM") as pslp,
            tc.tile_pool(name="oout", bufs=2) as opool,
            tc.tile_pool(name="stats", bufs=2) as spool,
        ):
            psl = pslp.tile([P, NQT], F32)
            for j in range(NQT):
                po = psO.tile([P, D], F32, tag="o")
                n_units = 2 * (j + 1)
                i = 0
                for kt in range(j + 1):
                    for p in range(2):
                        st = PT[:, p, _off(kt) + (j - kt) * P:
                                _off(kt) + (j - kt) * P + P]
                        for ec in range(D // 512):
                            nc.tensor.matmul(
                                po[:, ec * 512:(ec + 1) * 512], st,
                                v[:, p, kt, ec * 512:(ec + 1) * 512],
                                start=(i == 0), stop=(i == n_units - 1))
                        nc.tensor.matmul(
                            psl[:, j:j + 1], st, ones,
                            start=(i == 0), stop=(i == n_units - 1))
                        i += 1
                linv = spool.tile([P, 1], F32, tag="linv")
                nc.vector.reciprocal(linv, psl[:, j:j + 1])
                o_sb = opool.tile([P, D], F32, tag="o")
                for c in range(D // 512):
                    nc.vector.tensor_scalar_mul(
                        o_sb[:, c * 512:(c + 1) * 512],
                        po[:, c * 512:(c + 1) * 512], linv)
                    nc.sync.dma_start(
                        out[j * P:(j + 1) * P, c * 512:(c + 1) * 512],
                        o_sb[:, c * 512:(c + 1) * 512])


_PROG = None


def _get_prog():
    global _PROG
    if _PROG is None:
        nc = bacc.Bacc("TRN2", target_bir_lowering=False, debug=False,
                       enable_asserts=False)
        xq = nc.dram_tensor("xq", (D, QL), BF16, kind="ExternalInput").ap()
        wq = nc.dram_tensor("wq", (D, D), BF16, kind="ExternalInput").ap()
        wk = nc.dram_tensor("wk", (D, D), BF16, kind="ExternalInput").ap()
        wv = nc.dram_tensor("wv", (D, D), BF16, kind="ExternalInput").ap()
        mask = nc.dram_tensor("mask", (P, 256), F32, kind="ExternalInput").ap()
        out = nc.dram_tensor("out", (QL, D), F32, kind="ExternalOutput").ap()
        with tile.TileContext(nc) as tc:
            _body(tc, xq, wq, wk, wv, mask, out)
        nc.compile()
        _PROG = nc
    return _PROG


def _mask_np(h):
    """[k, q]-layout additive mask: block 0 = OWN slot, block 1 = PARTNER.

    Own slot: transposed causal triangle (keep k <= q) on the diagonal
    tile.  Partner slot diagonal tile: h=0's partner keys are ABOVE the
    diagonal (all masked); h=1's are below (all kept).
    """
    r = np.arange(P)[:, None]   # k (partition)
    c = np.arange(P)[None, :]   # q (free)
    tri = np.where(r <= c, 0.0, NEG).astype(np.float32)
    m = np.zeros((P, 256), np.float32)
    m[:, 0:P] = tri
    if h == 0:
        m[:, P:] = NEG
    return m


def _in_map_for_core(inputs, core):
    b, h = core // 2, core % 2
    xb = np.asarray(inputs["x"], np.float32)[b]
    xqb = xb.reshape(NQT, 2, P, D)[:, h].reshape(QL, D)
    bf = ml_dtypes.bfloat16
    return {
        "xq": np.ascontiguousarray(xqb.T.astype(bf)),
        "wq": np.ascontiguousarray(np.asarray(inputs["wq"]).astype(bf)),
        "wk": np.ascontiguousarray(np.asarray(inputs["wk"]).astype(bf)),
        "wv": np.ascontiguousarray(np.asarray(inputs["wv"]).astype(bf)),
        "mask": _mask_np(h),
    }


def _run(inputs, trace=False, tmpdir=None):
    nc = _get_prog()
    in_maps = [_in_map_for_core(inputs, c) for c in range(NCORES)]
    try:
        res = run_bass_kernel_spmd(nc, in_maps, core_ids=list(range(NCORES)),
                                   trace=trace, tmpdir=tmpdir)
    except Exception:
        # first execution of a fresh NEFF occasionally trips a transient
        # device error on this stack; one retry has always succeeded
        res = run_bass_kernel_spmd(nc, in_maps, core_ids=list(range(NCORES)),
                                   trace=trace, tmpdir=tmpdir)
    outf = np.empty((B, S, D), np.float32)
    for core in range(NCORES):
        b, h = core // 2, core % 2
        o = np.asarray(res.results[core]["out"], np.float32)
        outf[b].reshape(NQT, 2, P, D)[:, h] = o.reshape(NQT, P, D)
    return outf, res


def kernel(x, wq, wk, wv):
    outf, _ = _run({"x": x, "wq": wq, "wk": wk, "wv": wv}, trace=False)
    return outf
